# revision 13
# baseline (speedup 1.0000x reference)
"""Trainium2 Bass kernel for nn_Contextualizer (sparse_attention).

Per-core computation (data-parallel over batch B=8 across 8 NeuronCores):
    x0, x1 = split(x, 2, axis=-1)            # [N, D] each, N=2048, D=1024
    xn = x0 / sqrt(sum(x0^2, -1) + eps)      # row-normalize
    cosim = xn @ xn.T                        # [N, N], symmetric
    attn = tril(spatial_proj) * cosim
    out = (attn @ x0) * x1

The tril mask kills the upper triangle, so only lower-triangle tiles are
computed (~half the matmul work).  Gram formulation: G = x0 @ x0.T, with
the two 1/norm factors folded into existing elementwise stages:
    attnT[m, n] = maskT[m, n] * G[m, n] * s[m]      (mask stage, m on partitions)
    out[n, d]   = (ctx'[n, d] * s[n]) * x1[n, d]    (gating stage, n on partitions)
where s = 1/sqrt(sum x0^2 + eps), maskT = tril(spatial_proj).T (host-side),
and attnT (= attn transposed, via G symmetry) feeds the second matmul as
stationary operand: ctx'[n, d] = sum_m attnT[m, n] * x0[m, d].
"""

import numpy as np

B = 8
N = 2048
D = 1024
P = 128
NT = N // P      # 16 row tiles
DK = D // P      # 8 contraction tiles for matmul1
NJ_W = 256       # matmul1 free-dim chunk (n)
NJS = N // NJ_W  # 8
EPS = 1e-8

_NC_CACHE = {}


def _build():
    import concourse.bass as bass
    from concourse import bacc, mybir
    from concourse.tile import TileContext
    from concourse.masks import make_identity

    f32 = mybir.dt.float32
    f32r = mybir.dt.float32r
    bf16 = mybir.dt.bfloat16
    AF = mybir.ActivationFunctionType
    OP = mybir.AluOpType

    nc = bacc.Bacc("TRN2", debug=False, num_devices=B)
    x_ext = nc.declare_dram_parameter("x", [N, 2 * D], f32, isOutput=False)
    m_ext = nc.declare_dram_parameter("maskT", [N, N], f32, isOutput=False)
    out_ext = nc.declare_dram_parameter("out", [N, D], f32, isOutput=True)

    with TileContext(nc) as tc:
        with (
            tc.tile_pool(name="big", bufs=1) as big,
            tc.tile_pool(name="sqp", bufs=3) as sqp,
            tc.tile_pool(name="astrip", bufs=3) as astrip,
            tc.tile_pool(name="maskp", bufs=16) as maskp,
            tc.tile_pool(name="gio", bufs=2) as gio,
            tc.tile_pool(name="outp", bufs=2) as outp,
            tc.tile_pool(name="pt", bufs=2, space="PSUM") as pt,
            tc.tile_pool(name="pa", bufs=3, space="PSUM") as pa,
            tc.tile_pool(name="pb", bufs=3, space="PSUM") as pb,
        ):
            x0b = big.tile([P, NT, D], bf16)       # x0, natural [m, d] layout
            x0T = big.tile([P, DK, N], bf16)       # x0 transposed: [d, n]
            ident = big.tile([P, P], bf16)
            make_identity(nc, ident)
            nrm2 = big.tile([P, NT], f32)
            scal = big.tile([P, NT], f32)
            tmpa = big.tile([P, NT], f32)
            tmpb = big.tile([P, NT], f32)

            def setup_tile(i):
                """DMA x0 tile i; transpose raw x0 into x0T; row sum-squares."""
                x0f = sqp.tile([P, D], f32, tag="x0f")
                nc.sync.dma_start(x0f[:], x_ext.ap()[i * P : (i + 1) * P, 0:D])
                nc.vector.tensor_copy(x0b[:, i, :], x0f[:])
                ps = pt.tile([P, DK, P], bf16)
                for dk in range(DK):
                    nc.tensor.transpose(
                        ps[:, dk, :], x0b[:, i, dk * P : (dk + 1) * P], ident
                    )
                nc.scalar.copy(x0T[:, :, i * P : (i + 1) * P], ps[:])
                sq = sqp.tile([P, D], f32, tag="sq")
                nc.scalar.activation(
                    sq[:], x0f[:], AF.Square, accum_out=nrm2[:, i : i + 1]
                )

            def stats_pair(i0, i1):
                """scal[:, i0:i1+1] = rsqrt(nrm2 + EPS), one Newton step."""
                sl = slice(i0, i1 + 1)
                nc.gpsimd.tensor_scalar_add(tmpa[:, sl], nrm2[:, sl], EPS)
                nc.scalar.activation(tmpb[:, sl], tmpa[:, sl], AF.Sqrt)
                nc.vector.reciprocal(scal[:, sl], tmpb[:, sl])
                nc.gpsimd.tensor_mul(tmpb[:, sl], scal[:, sl], scal[:, sl])
                nc.gpsimd.tensor_mul(tmpb[:, sl], tmpb[:, sl], tmpa[:, sl])
                nc.gpsimd.tensor_scalar(
                    tmpb[:, sl], tmpb[:, sl], -0.5, 1.5, op0=OP.mult, op1=OP.add
                )
                nc.gpsimd.tensor_mul(scal[:, sl], scal[:, sl], tmpb[:, sl])

            def issue_masks(nj):
                """Prefetch all mask tiles for strip nj."""
                n0 = nj * NJ_W
                n_mtiles = min(2 * nj + 2, NT)
                tiles = []
                for mi2 in range(0, n_mtiles, 2):
                    mt = maskp.tile([P, 2, NJ_W], f32, tag="mt")
                    nc.sync.dma_start(
                        mt[:],
                        m_ext.ap()[
                            mi2 * P : (mi2 + 2) * P, n0 : n0 + NJ_W
                        ].rearrange("(c p) n -> p c n", p=P),
                    )
                    tiles.append(mt)
                return tiles

            def phase_a(nj, masks):
                """attnT strip for n-chunk nj: tiles mi = 0..2nj+1."""
                n0 = nj * NJ_W
                n_mtiles = min(2 * nj + 2, NT)
                A = astrip.tile([P, NT, NJ_W], bf16, tag="A")
                for mi2 in range(0, n_mtiles, 2):
                    mt = masks[mi2 // 2]
                    for c in range(2):
                        mi = mi2 + c
                        pcs = pa.tile([P, NJ_W], f32)
                        for dk in range(DK):
                            nc.tensor.matmul(
                                pcs[:],
                                x0T[:, dk, mi * P : (mi + 1) * P],
                                x0T[:, dk, n0 : n0 + NJ_W],
                                start=(dk == 0),
                                stop=(dk == DK - 1),
                            )
                        nc.vector.scalar_tensor_tensor(
                            out=A[:, mi, :],
                            in0=pcs[:],
                            scalar=scal[:, mi : mi + 1],
                            in1=mt[:, c, :],
                            op0=OP.mult,
                            op1=OP.mult,
                        )
                return A

            def phase_b(nj, A):
                """ctx rows for n-tiles 2nj, 2nj+1; scale+gate with x1; DMA out."""
                for sub in range(2):
                    ni = 2 * nj + sub
                    off = sub * P
                    x1t = gio.tile([P, D], f32, tag="x1t")
                    nc.scalar.dma_start(
                        x1t[:], x_ext.ap()[ni * P : (ni + 1) * P, D : 2 * D]
                    )
                    ot = outp.tile([P, D], f32, tag="ot")
                    for dc in range(2):
                        pob = pb.tile([P, 512], f32, tag="pob")
                        for mi in range(ni + 1):
                            nc.tensor.matmul(
                                pob[:],
                                A[:, mi, off : off + P],
                                x0b[:, mi, dc * 512 : (dc + 1) * 512],
                                start=(mi == 0),
                                stop=(mi == ni),
                            )
                        nc.vector.scalar_tensor_tensor(
                            out=ot[:, dc * 512 : (dc + 1) * 512],
                            in0=pob[:],
                            scalar=scal[:, ni : ni + 1],
                            in1=x1t[:, dc * 512 : (dc + 1) * 512],
                            op0=OP.mult,
                            op1=OP.mult,
                        )
                    nc.scalar.dma_start(
                        out_ext.ap()[ni * P : (ni + 1) * P, :], ot[:]
                    )

            # Software-pipelined emission: setup tiles arrive just before the
            # attnT strip that first needs them; phase_b trails by one strip.
            prev_A = None
            masks_cur = issue_masks(0)
            for nj in range(NJS):
                setup_tile(2 * nj)
                setup_tile(2 * nj + 1)
                stats_pair(2 * nj, 2 * nj + 1)
                masks_next = issue_masks(nj + 1) if nj + 1 < NJS else None
                A = phase_a(nj, masks_cur)
                if prev_A is not None:
                    phase_b(nj - 1, prev_A)
                prev_A = A
                masks_cur = masks_next
            phase_b(NJS - 1, prev_A)

    nc.compile()
    return nc


def _get_nc():
    if "nc" not in _NC_CACHE:
        _NC_CACHE["nc"] = _build()
    return _NC_CACHE["nc"]


def _run(x, spatial_proj, trace=False):
    from concourse.bass_utils import run_bass_kernel_spmd

    nc = _get_nc()
    x = np.ascontiguousarray(np.asarray(x, dtype=np.float32))
    sp = np.asarray(spatial_proj, dtype=np.float32)
    maskT = np.ascontiguousarray(np.tril(sp).T)
    in_maps = [
        {"x": np.ascontiguousarray(x[b]), "maskT": maskT} for b in range(B)
    ]
    res = run_bass_kernel_spmd(
        nc, in_maps, core_ids=list(range(B)), trace=trace
    )
    out = np.stack([res.results[b]["out"] for b in range(B)], axis=0)
    return out.astype(np.float32), res


def kernel(x, spatial_proj):
    out, _ = _run(x, spatial_proj, trace=False)
    return out


if __name__ == "__main__":
    rng = np.random.default_rng(0)
    x = rng.standard_normal((B, N, 2 * D), dtype=np.float32)
    sp = (rng.standard_normal((N, N), dtype=np.float32) * np.sqrt(1.0 / N)).astype(
        np.float32
    )
    out = kernel(x, sp)
    print("out shape", out.shape, out.dtype)


# revision 14
# speedup vs baseline: 1.0824x; 1.0824x over previous
"""Trainium2 Bass kernel for nn_Contextualizer (sparse_attention).

Per-core computation (data-parallel over batch B=8 across 8 NeuronCores):
    x0, x1 = split(x, 2, axis=-1)            # [N, D] each, N=2048, D=1024
    xn = x0 / sqrt(sum(x0^2, -1) + eps)      # row-normalize
    cosim = xn @ xn.T                        # [N, N], symmetric
    attn = tril(spatial_proj) * cosim
    out = (attn @ x0) * x1

The tril mask kills the upper triangle, so only lower-triangle tiles are
computed (~half the matmul work).  Gram formulation: G = x0 @ x0.T, with
the two 1/norm factors folded into existing elementwise stages:
    attnT[m, n] = maskT[m, n] * G[m, n] * s[m]      (mask stage, m on partitions)
    out[n, d]   = (ctx'[n, d] * s[n]) * x1[n, d]    (gating stage, n on partitions)
where s = 1/sqrt(sum x0^2 + eps), maskT = tril(spatial_proj).T (host-side),
and attnT (= attn transposed, via G symmetry) feeds the second matmul as
stationary operand: ctx'[n, d] = sum_m attnT[m, n] * x0[m, d].
"""

import numpy as np

B = 8
N = 2048
D = 1024
P = 128
NT = N // P      # 16 row tiles
DK = D // P      # 8 contraction tiles for matmul1
NJ_W = 256       # matmul1 free-dim chunk (n)
NJS = N // NJ_W  # 8
EPS = 1e-8

_NC_CACHE = {}


def _build():
    import concourse.bass as bass
    from concourse import bacc, mybir
    from concourse.tile import TileContext
    from concourse.masks import make_identity

    f32 = mybir.dt.float32
    f32r = mybir.dt.float32r
    bf16 = mybir.dt.bfloat16
    AF = mybir.ActivationFunctionType
    OP = mybir.AluOpType

    nc = bacc.Bacc("TRN2", debug=False, num_devices=B)
    x_ext = nc.declare_dram_parameter("x", [N, 2 * D], f32, isOutput=False)
    m_ext = nc.declare_dram_parameter("maskT", [N, N], f32, isOutput=False)
    out_ext = nc.declare_dram_parameter("out", [N, D], f32, isOutput=True)

    with TileContext(nc) as tc:
        with (
            tc.tile_pool(name="big", bufs=1) as big,
            tc.tile_pool(name="sqp", bufs=3) as sqp,
            tc.tile_pool(name="astrip", bufs=3) as astrip,
            tc.tile_pool(name="maskp", bufs=4) as maskp,
            tc.tile_pool(name="gio", bufs=2) as gio,
            tc.tile_pool(name="outp", bufs=2) as outp,
            tc.tile_pool(name="pt", bufs=2, space="PSUM") as pt,
            tc.tile_pool(name="pa", bufs=3, space="PSUM") as pa,
            tc.tile_pool(name="pb", bufs=3, space="PSUM") as pb,
        ):
            x0b = big.tile([P, NT, D], bf16)       # x0, natural [m, d] layout
            x0T = big.tile([P, DK, N], bf16)       # x0 transposed: [d, n]
            ident = big.tile([P, P], bf16)
            make_identity(nc, ident)
            nrm2 = big.tile([P, NT], f32)
            scal = big.tile([P, NT], f32)
            tmpa = big.tile([P, NT], f32)
            tmpb = big.tile([P, NT], f32)

            def setup_dma(i):
                x0f = sqp.tile([P, D], f32, tag="x0f")
                nc.sync.dma_start(x0f[:], x_ext.ap()[i * P : (i + 1) * P, 0:D])
                return x0f

            def setup_compute(i, x0f):
                """Cast to bf16; transpose raw x0 into x0T; row sum-squares."""
                nc.scalar.copy(x0b[:, i, :], x0f[:])
                ps = pt.tile([P, DK, P], bf16)
                for dk in range(DK):
                    nc.tensor.transpose(
                        ps[:, dk, :], x0b[:, i, dk * P : (dk + 1) * P], ident
                    )
                nc.scalar.copy(x0T[:, :, i * P : (i + 1) * P], ps[:])
                sq = sqp.tile([P, D], f32, tag="sq")
                nc.scalar.activation(
                    sq[:], x0f[:], AF.Square, accum_out=nrm2[:, i : i + 1]
                )

            def stats_pair(i0, i1):
                """scal[:, i0:i1+1] = rsqrt(nrm2 + EPS), one Newton step."""
                sl = slice(i0, i1 + 1)
                nc.gpsimd.tensor_scalar_add(tmpa[:, sl], nrm2[:, sl], EPS)
                nc.scalar.activation(tmpb[:, sl], tmpa[:, sl], AF.Ln)
                nc.scalar.activation(scal[:, sl], tmpb[:, sl], AF.Exp, scale=-0.5)
                nc.gpsimd.tensor_mul(tmpb[:, sl], scal[:, sl], scal[:, sl])
                nc.gpsimd.tensor_mul(tmpb[:, sl], tmpb[:, sl], tmpa[:, sl])
                nc.gpsimd.tensor_scalar(
                    tmpb[:, sl], tmpb[:, sl], -0.5, 1.5, op0=OP.mult, op1=OP.add
                )
                nc.gpsimd.tensor_mul(scal[:, sl], scal[:, sl], tmpb[:, sl])

            def phase_a(nj):
                """attnT strip for n-chunk nj: tiles mi = 0..2nj+1."""
                n0 = nj * NJ_W
                n_mtiles = min(2 * nj + 2, NT)
                A = astrip.tile([P, NT, NJ_W], bf16, tag="A")
                for mi2 in range(0, n_mtiles, 2):
                    mt = maskp.tile([P, 2, NJ_W], f32, tag="mt")
                    nc.sync.dma_start(
                        mt[:],
                        m_ext.ap()[
                            mi2 * P : (mi2 + 2) * P, n0 : n0 + NJ_W
                        ].rearrange("(c p) n -> p c n", p=P),
                    )
                    for c in range(2):
                        mi = mi2 + c
                        pcs = pa.tile([P, NJ_W], f32)
                        for dk in range(DK):
                            nc.tensor.matmul(
                                pcs[:],
                                x0T[:, dk, mi * P : (mi + 1) * P],
                                x0T[:, dk, n0 : n0 + NJ_W],
                                start=(dk == 0),
                                stop=(dk == DK - 1),
                            )
                        nc.vector.scalar_tensor_tensor(
                            out=A[:, mi, :],
                            in0=pcs[:],
                            scalar=scal[:, mi : mi + 1],
                            in1=mt[:, c, :],
                            op0=OP.mult,
                            op1=OP.mult,
                        )
                return A

            def phase_b(nj, A):
                """ctx rows for n-tiles 2nj, 2nj+1; scale+gate with x1; DMA out."""
                for sub in range(2):
                    ni = 2 * nj + sub
                    off = sub * P
                    x1t = gio.tile([P, D], f32, tag="x1t")
                    nc.scalar.dma_start(
                        x1t[:], x_ext.ap()[ni * P : (ni + 1) * P, D : 2 * D]
                    )
                    ot = outp.tile([P, D], f32, tag="ot")
                    for dc in range(2):
                        pob = pb.tile([P, 512], f32, tag="pob")
                        for mi in range(ni + 1):
                            nc.tensor.matmul(
                                pob[:],
                                A[:, mi, off : off + P],
                                x0b[:, mi, dc * 512 : (dc + 1) * 512],
                                start=(mi == 0),
                                stop=(mi == ni),
                            )
                        nc.vector.scalar_tensor_tensor(
                            out=ot[:, dc * 512 : (dc + 1) * 512],
                            in0=pob[:],
                            scalar=scal[:, ni : ni + 1],
                            in1=x1t[:, dc * 512 : (dc + 1) * 512],
                            op0=OP.mult,
                            op1=OP.mult,
                        )
                    nc.scalar.dma_start(
                        out_ext.ap()[ni * P : (ni + 1) * P, :], ot[:]
                    )

            # Software-pipelined emission: setup tiles arrive just before the
            # attnT strip that first needs them; phase_b trails by one strip.
            prev_A = None
            f0 = setup_dma(0)
            f1 = setup_dma(1)
            setup_compute(0, f0)
            setup_compute(1, f1)
            stats_pair(0, 1)
            for nj in range(NJS):
                if nj + 1 < NJS:
                    fa = setup_dma(2 * nj + 2)
                    fb = setup_dma(2 * nj + 3)
                A = phase_a(nj)
                if nj + 1 < NJS:
                    setup_compute(2 * nj + 2, fa)
                    setup_compute(2 * nj + 3, fb)
                    stats_pair(2 * nj + 2, 2 * nj + 3)
                if prev_A is not None:
                    phase_b(nj - 1, prev_A)
                prev_A = A
            phase_b(NJS - 1, prev_A)

    nc.compile()
    return nc


def _get_nc():
    if "nc" not in _NC_CACHE:
        _NC_CACHE["nc"] = _build()
    return _NC_CACHE["nc"]


def _run(x, spatial_proj, trace=False):
    from concourse.bass_utils import run_bass_kernel_spmd

    nc = _get_nc()
    x = np.ascontiguousarray(np.asarray(x, dtype=np.float32))
    sp = np.asarray(spatial_proj, dtype=np.float32)
    maskT = np.ascontiguousarray(np.tril(sp).T)
    in_maps = [
        {"x": np.ascontiguousarray(x[b]), "maskT": maskT} for b in range(B)
    ]
    res = run_bass_kernel_spmd(
        nc, in_maps, core_ids=list(range(B)), trace=trace
    )
    out = np.stack([res.results[b]["out"] for b in range(B)], axis=0)
    return out.astype(np.float32), res


def kernel(x, spatial_proj):
    out, _ = _run(x, spatial_proj, trace=False)
    return out


if __name__ == "__main__":
    rng = np.random.default_rng(0)
    x = rng.standard_normal((B, N, 2 * D), dtype=np.float32)
    sp = (rng.standard_normal((N, N), dtype=np.float32) * np.sqrt(1.0 / N)).astype(
        np.float32
    )
    out = kernel(x, sp)
    print("out shape", out.shape, out.dtype)
